# revision 44
# baseline (speedup 1.0000x reference)
"""Trainium2 Bass kernel for nn_CriticNetwork (gnn_message_passing).

Mathematical simplification (verified against the reference): the
reference broadcasts edge_index to (B, 2, E) and reshapes to
(2, B*E); row-major reshape makes src == dst elementwise, so every
edge is a self-edge and with GCN normalization both GCNConv layers
collapse to plain linear layers.  Since the post-relu node/col heads
are linear, the device only needs per-core SUMS of the hidden
activations; the host applies the tiny final heads.

v2 design (vs the 35.5us v1 baseline) — driven by NTFF trace analysis:
  * fp8(e4m3) x over the wire (1.6MB/core instead of 3.2MB bf16) and
    fp8 W1 with DoubleRow matmuls: contraction 2x128 packs FOUR
    64-feature nodes per moving column pair, halving L1 PE time.
    (W2/h1 stay bf16: measured node_avg rel-err 4e-4, budget 2e-2.)
  * Row-shaped output [2,128] fp32 via an fp32 PE transpose
    (stats x I128): the v1 [128,1] column output emitted 128 4-byte
    HBM descriptors -> read-modify-write grind, ~6us of tail.
  * Few, large DMAs on three queues (sync HWDGE / scalar HWDGE /
    gpsimd SWDGE): v1's 7 chunk DMAs on one queue ran at 183GB/s.
  * Minimal instruction count: measured ~255ns dispatch overhead per
    (dependent) instruction; v1 had 522 instructions.

Per-core layout (25000 nodes): 7 psum chunks (6 x 512 cols + 53).
Each psum column holds 8 nodes (128 rows = 8 nodes x 16 hidden).
Chunk x layout [128, 4, M]: blocks (A0,A1,B0,B1); DR matmul A
consumes blocks 0-1 -> psum rows 0:64, B -> rows 64:128.
x[p, hb, m] = feat (p%64) of node(chunk_base + (2*hb + p//64)*M + m).
"""

import ml_dtypes
import numpy as np

import concourse.bacc as bacc
import concourse.mybir as mybir
import concourse.tile as tile
from concourse.bass_utils import run_bass_kernel_spmd

P = 128
N_CORES = 8
B, N, F_NODE, H = 4, 50000, 64, 16
C, F_COL = 1000, 32
NODES_PER_CORE = (B * N) // N_CORES          # 25000
COLN = (B * C) // N_CORES                    # 500 col rows per core

MM = 512                                     # psum bank cols (fp32)
NODES_PER_CHUNK = 8 * MM                     # 4096
N_FULL = NODES_PER_CORE // NODES_PER_CHUNK   # 6 full chunks
# tail: 424 nodes -> 53 cols, padded to 64 (DoubleRow AP needs the
# k-pair stride %16 == 0 and an even column count); pad nodes are
# zero and the host subtracts their bias-path contribution.
M_TAIL = 64
N_PAD = 8 * M_TAIL - (NODES_PER_CORE - N_FULL * NODES_PER_CHUNK)    # 88
CHUNK_M = [MM] * N_FULL + [M_TAIL]           # 7 chunks
N_CHUNKS = len(CHUNK_M)

# wpack (bf16): just blockdiag(W2 x8) [128, 128]
NWP = P
# wcol (bf16): col_W1 [32, 0:16] + colT [32, 16:516] -- shipped late on
# the sync queue so the scheduler cannot hoist the col matmul ahead of
# the node chunks on the PE.
CW1_OFF = 0
COLT_OFF = H
NWC = COLT_OFF + COLN                        # 516

# bias32 (fp32, tiny, first on the scalar queue -- relu1 needs b1)
B1_OFF = 0                                   # [128, 1] b1 tiled x8
B2_OFF = 1                                   # [128, 1] b2 tiled x8
NB2_OFF = 2                                  # [128, 1] -b2 tiled x8
CB1_OFF = 3                                  # [16, 1]  col_b1
NB32 = 4

DT = mybir.dt.bfloat16
FP8 = mybir.dt.float8e4
NPBF = ml_dtypes.bfloat16
NPF8 = ml_dtypes.float8_e4m3                 # TRN FP8_EXP4-compatible
DR = mybir.MatmulPerfMode.DoubleRow

PROFILE = False
CHECK_WAITS = True
LAST_EXEC_TIME_NS = None
LAST_RESULTS = None

_NC_CACHE = {}


def _build_nc():
    f32 = mybir.dt.float32
    Relu = mybir.ActivationFunctionType.Relu
    nc = bacc.Bacc("TRN2")

    # Two full-partition DoubleRow stationaries (the ISA rejects
    # partition-sliced PSUM outputs in DR mode): w1A covers x blocks
    # 0-1 -> psum rows 0:64 (cols 64:128 zero), w1B covers blocks
    # 2-3 -> rows 64:128; the two matmuls accumulate into one bank.
    # w1A|w1B|chunk0 ride ONE first transfer (xw0) so the first compute
    # waits a single DMA-completion round trip.
    xw0 = nc.dram_tensor("xw0", [P, 2, 2 * P + 2 * MM], FP8,
                         kind="ExternalInput")
    xt1 = nc.dram_tensor("xt1", [P, 8, MM], FP8, kind="ExternalInput")
    xt2 = nc.dram_tensor("xt2", [P, 8, MM], FP8, kind="ExternalInput")
    xt3 = nc.dram_tensor("xt3", [P, 4, MM], FP8, kind="ExternalInput")
    xt4 = nc.dram_tensor("xt4", [P, 4, M_TAIL], FP8, kind="ExternalInput")
    wpack = nc.dram_tensor("wpack", [P, NWP], DT, kind="ExternalInput")
    wcol = nc.dram_tensor("wcol", [F_COL, NWC], DT, kind="ExternalInput")
    bias32 = nc.dram_tensor("bias32", [P, NB32], f32, kind="ExternalInput")
    ident32 = nc.dram_tensor("ident32", [P, P], f32, kind="ExternalInput")
    out_acc = nc.dram_tensor("out_acc", [N_CHUNKS + 1, P], f32,
                             kind="ExternalOutput")

    with tile.TileContext(nc) as tc:
        with (
            tc.tile_pool(name="consts", bufs=1) as consts,
            tc.tile_pool(name="xin", bufs=1) as xin,
            tc.tile_pool(name="work", bufs=1) as work,
            tc.tile_pool(name="psum", bufs=1, space="PSUM") as psum,
        ):
            # --- input DMAs: ONE queue, strict need-order -------------
            # Parallel queues round-robin per packet, so they dilute
            # each other and the first-needed transfer lands almost as
            # late as the last.  A single FIFO queue completes in
            # consumption order at full bandwidth instead.
            # sync queue: the bulk x stream only, strict need-order (a
            # second bulk queue dilutes packet round-robin and delays
            # the first-needed transfer; constants ride scalar instead).
            x0 = xin.tile([P, 2, 2 * P + 2 * MM], FP8, tag="x0", name="x0")
            nc.sync.dma_start(x0[:, :, :], xw0[:, :, :])
            x1 = xin.tile([P, 8, MM], FP8, tag="x1", name="x1")
            nc.sync.dma_start(x1[:, :, :], xt1[:, :, :])
            x2 = xin.tile([P, 8, MM], FP8, tag="x2", name="x2")
            nc.sync.dma_start(x2[:, :, :], xt2[:, :, :])
            x3 = xin.tile([P, 4, MM], FP8, tag="x3", name="x3")
            nc.sync.dma_start(x3[:, :, :], xt3[:, :, :])
            x4 = xin.tile([P, 4, M_TAIL], FP8, tag="x4", name="x4")
            nc.sync.dma_start(x4[:, :, :], xt4[:, :, :])
            bs = consts.tile([P, NB32], f32)
            nc.scalar.dma_start(bs[:], bias32[:])
            wp = consts.tile([P, NWP], DT)
            nc.scalar.dma_start(wp[:], wpack[:])
            # +1 spare col: DVE touches it mid-loop so the scheduler
            # cannot hoist the col matmul (which reads this tile) ahead
            # of the node chunks.
            wc = consts.tile([F_COL, NWC + 1], DT)
            nc.scalar.dma_start(wc[:, 0:NWC], wcol[:])
            nc.vector.memset(wc[:, NWC:NWC + 1], 0.0)
            i128t = consts.tile([P, P], f32)
            nc.scalar.dma_start(i128t[:], ident32[:])
            w1s = x0  # stationaries live in the head of xw0

            w2_t = wp[:, 0:P]
            cw1_t = wc[:, CW1_OFF:CW1_OFF + H]
            colT_t = wc[:, COLT_OFF:COLT_OFF + COLN]
            i128 = i128t[:, :]
            b1_t = bs[:, B1_OFF:B1_OFF + 1]
            b2_t = bs[:, B2_OFF:B2_OFF + 1]
            nb2_t = bs[:, NB2_OFF:NB2_OFF + 1]
            cb1_t = bs[:H, CB1_OFF:CB1_OFF + 1]

            # stats: col c = chunk-c row sums; col N_CHUNKS = col-path
            # totals (rows 0-15; rest zeroed).  The whole tile is
            # transposed out at the end; the host sums the chunk rows.
            stats = work.tile([P, N_CHUNKS + 1], f32)
            nc.vector.memset(stats[:, N_CHUNKS:N_CHUNKS + 1], 0.0)

            NBUF = 3
            ps1_t = [psum.tile([P, MM], f32, tag=f"ps1_{k}", name=f"ps1_{k}")
                     for k in range(NBUF)]
            ps2_t = [psum.tile([P, MM], f32, tag=f"ps2_{k}", name=f"ps2_{k}")
                     for k in range(NBUF)]
            h1_t = [work.tile([P, MM], DT, tag=f"h1_{k}", name=f"h1_{k}")
                    for k in range(NBUF)]
            scr_t = [work.tile([P, MM], DT, tag=f"scr_{k}", name=f"scr_{k}")
                     for k in range(NBUF)]

            # chunk -> (rhs slice for matmul A, rhs slice for matmul B)
            W0 = 2 * P  # chunk0 data offset inside xw0
            srcs = [
                (x0[:, 0:2, W0:W0 + MM], x0[:, 0:2, W0 + MM:W0 + 2 * MM]),
                (x1[:, 0:2, :], x1[:, 2:4, :]),
                (x1[:, 4:6, :], x1[:, 6:8, :]),
                (x2[:, 0:2, :], x2[:, 2:4, :]),
                (x2[:, 4:6, :], x2[:, 6:8, :]),
                (x3[:, 0:2, :], x3[:, 2:4, :]),
                (x4[:, 0:2, :], x4[:, 2:4, :]),
            ]

            for c, (rhsA, rhsB) in enumerate(srcs):
                M = CHUNK_M[c]
                ps1 = ps1_t[c % NBUF]
                nc.tensor.matmul(
                    ps1[:, :M], w1s[:, :, 0:P], rhsA,
                    start=True, stop=False, perf_mode=DR)
                nc.tensor.matmul(
                    ps1[:, :M], w1s[:, :, P:2 * P], rhsB,
                    start=False, stop=True, perf_mode=DR)
                h1 = h1_t[c % NBUF]
                nc.scalar.activation(h1[:, :M], ps1[:, :M], Relu, bias=b1_t)
                ps2 = ps2_t[c % NBUF]
                nc.tensor.matmul(ps2[:, :M], w2_t, h1[:, :M],
                                 start=True, stop=True)
                scr = scr_t[c % NBUF]
                # relu(x + b2) = max(x, -b2) + b2 -- the DVE's second ALU
                # stage does not apply `max`, so keep max in stage 0.
                nc.vector.tensor_scalar(
                    scr[:, :M], ps2[:, :M], nb2_t, b2_t,
                    mybir.AluOpType.max, mybir.AluOpType.add,
                    accum_out=stats[:, c:c + 1])
                if c == 2:
                    # pin the col matmul behind chunk 2: write the spare
                    # col its rhs covers (junk col excluded from accum).
                    nc.vector.tensor_copy(wc[0:1, NWC:NWC + 1],
                                          stats[0:1, 2:3])

            # column-features path: h = relu(colT.T @ col_W1 + col_b1).
            # rhs covers one junk col (the scheduling pin); the accum
            # reads only the first COLN psum columns.
            psc = psum.tile([H, COLN + 1], f32, tag="psc")
            nc.tensor.matmul(psc[:, :], cw1_t,
                             wc[:, COLT_OFF:COLT_OFF + COLN + 1],
                             start=True, stop=True)
            colscr = work.tile([H, COLN], f32)
            nc.scalar.activation(colscr[:], psc[:, 0:COLN], Relu, bias=cb1_t,
                                 accum_out=stats[:H, N_CHUNKS:N_CHUNKS + 1])

            # transpose all stats columns -> [8, 128] rows on the PE
            ptr = psum.tile([N_CHUNKS + 1, P], f32, tag="ptr")
            nc.tensor.transpose(ptr[:, :], stats[:, :], i128)
            row = work.tile([N_CHUNKS + 1, P], f32)
            nc.vector.tensor_copy(row[:], ptr[:])
            nc.sync.dma_start(out_acc[:], row[:])

    nc.finalize()

    if CHECK_WAITS:
        for blk in nc.m.functions[0].blocks:
            for inst in blk.instructions:
                si = inst.sync_info
                nwait = len(si.on_wait) if si and si.on_wait else 0
                limit = 2 if type(inst).__name__ in (
                    "InstEventSemaphore", "InstDrain", "InstDMACopy") else 1
                assert nwait <= limit, (
                    inst.name, type(inst).__name__,
                    [w.ant_name for w in si.on_wait])
    return nc


def _get_nc():
    if "nc" not in _NC_CACHE:
        _NC_CACHE["nc"] = _build_nc()
    return _NC_CACHE["nc"]


def _pack_x_core(xc):
    """xc [25000, 64] f32 -> [128, 12544] fp8 in chunked DR layout."""
    if N_PAD:
        xc = np.concatenate(
            [xc, np.zeros((N_PAD, F_NODE), xc.dtype)], axis=0)
    cols = []
    base = 0
    for M in CHUNK_M:
        nodes = xc[base:base + 8 * M]                 # [8M, 64]
        a = nodes.reshape(4, 2, M, F_NODE)            # (hb, prow, m, feat)
        cols.append(a.transpose(1, 3, 0, 2).reshape(P, 4 * M))
        base += 8 * M
    return np.concatenate(cols, axis=1)


def _prep_in_maps(node_features, col_features, W1, b1, W2, b2, col_W1, col_b1):
    f32 = np.float32
    x = np.ascontiguousarray(node_features, dtype=f32).reshape(B * N, F_NODE)
    colf = np.ascontiguousarray(col_features, dtype=f32).reshape(B * C, F_COL)
    W1 = np.asarray(W1, f32)
    W2 = np.asarray(W2, f32)

    # DoubleRow stationaries: w1X[p, k, 16q+f] = W1[p%64, f] where
    # q = 2*k + p//64 (4 nodes per moving column pair); w1A fills
    # out rows 0:64 (blocks 0-1), w1B rows 64:128 (blocks 2-3).
    w1dr = np.zeros((P, 2, 2 * P), f32)
    for k in range(2):
        for ph in range(2):
            q = 2 * k + ph
            w1dr[ph * 64:(ph + 1) * 64, k, 16 * q:16 * q + H] = W1          # A
            w1dr[ph * 64:(ph + 1) * 64, k, P + 64 + 16 * q:P + 64 + 16 * q + H] = W1  # B
    w1dr = w1dr.astype(NPF8)

    wpack = np.zeros((P, NWP), f32)
    for g in range(P // H):
        wpack[H * g:H * g + H, H * g:H * g + H] = W2
    wpack = wpack.astype(NPBF)

    wcol_base = np.zeros((F_COL, NWC), f32)
    wcol_base[:, CW1_OFF:CW1_OFF + H] = np.asarray(col_W1, f32)

    bias32 = np.zeros((P, NB32), f32)
    bias32[:, B1_OFF] = np.tile(np.asarray(b1, f32), P // H)
    bias32[:, B2_OFF] = np.tile(np.asarray(b2, f32), P // H)
    bias32[:, NB2_OFF] = -bias32[:, B2_OFF]
    bias32[:H, CB1_OFF] = np.asarray(col_b1, f32)
    ident32 = np.eye(P, dtype=f32)

    bounds = np.cumsum([0] + [4 * M for M in CHUNK_M])  # chunk col offsets

    in_maps = []
    for core in range(N_CORES):
        n0 = core * NODES_PER_CORE
        xp = _pack_x_core(x[n0:n0 + NODES_PER_CORE]).astype(NPF8)
        wcol = wcol_base.copy()
        wcol[:, COLT_OFF:COLT_OFF + COLN] = \
            colf[core * COLN:(core + 1) * COLN].T
        # xw0 [128, 2, 256 + 1024]: w1A | w1B | chunk0-A (blocks 0,1) |
        # chunk0-B (blocks 2,3); dim1 is the DoubleRow k index.
        c0 = xp[:, bounds[0]:bounds[1]].reshape(P, 4, MM)
        xw0 = np.concatenate(
            [w1dr, c0[:, 0:2, :], c0[:, 2:4, :]], axis=2)
        in_maps.append({
            "xw0": np.ascontiguousarray(xw0),
            "xt1": xp[:, bounds[1]:bounds[3]].reshape(P, 8, MM),
            "xt2": xp[:, bounds[3]:bounds[5]].reshape(P, 8, MM),
            "xt3": xp[:, bounds[5]:bounds[6]].reshape(P, 4, MM),
            "xt4": np.ascontiguousarray(
                xp[:, bounds[6]:bounds[7]].reshape(P, 4, M_TAIL)),
            "wpack": wpack,
            "wcol": wcol.astype(NPBF),
            "bias32": bias32,
            "ident32": ident32,
        })
    return in_maps


def kernel(node_features, col_features, edge_index, W1, b1, W2, b2,
           node_fc_W, node_fc_b, col_W1, col_b1, col_W2, col_b2,
           fc_W, fc_b, out_W, out_b):
    global LAST_EXEC_TIME_NS, LAST_RESULTS
    # edge_index provably does not affect the output (see module docstring).
    in_maps = _prep_in_maps(node_features, col_features,
                            W1, b1, W2, b2, col_W1, col_b1)
    nc = _get_nc()
    res = run_bass_kernel_spmd(nc, in_maps, core_ids=list(range(N_CORES)),
                               trace=PROFILE)
    LAST_EXEC_TIME_NS = res.exec_time_ns
    LAST_RESULTS = res
    outs = res.results

    node_fc_W = np.asarray(node_fc_W, np.float32)
    col_W2 = np.asarray(col_W2, np.float32)
    # Device relu2 computes max(x, -b2) (+b2 once per chunk-reduction),
    # i.e. each chunk col = sum(relu) - (M-1)*b2; add the constant back.
    # Zero-pad nodes contribute relu(W2.T relu(b1) + b2) each; subtract.
    b2f = np.asarray(b2, np.float32)
    summ1 = np.float32(sum(M - 1 for M in CHUNK_M))
    pad_h2 = np.maximum(
        np.maximum(np.asarray(b1, np.float32), 0.0) @ np.asarray(W2, np.float32)
        + b2f, 0.0) * np.float32(N_PAD) - summ1 * b2f * np.float32(P // H)
    node_avg = np.zeros((B, 1), np.float32)
    col_avg = np.zeros((B, 1), np.float32)
    for b in range(B):
        acc = (outs[2 * b]["out_acc"].astype(np.float32) +
               outs[2 * b + 1]["out_acc"].astype(np.float32))
        ns = acc[:N_CHUNKS].sum(axis=0).reshape(P // H, H).sum(axis=0) \
            - 2.0 * pad_h2
        cs = acc[N_CHUNKS, :H]                           # col feature sums
        node_avg[b, 0] = (ns / np.float32(N)) @ node_fc_W[:, 0] + \
            np.asarray(node_fc_b, np.float32)[0]
        col_avg[b, 0] = (cs / np.float32(C)) @ col_W2[:, 0] + \
            np.asarray(col_b2, np.float32)[0]

    combined = np.concatenate([node_avg, col_avg], axis=1)      # [B, 2]
    z = np.maximum(combined @ np.asarray(fc_W, np.float32) +
                   np.asarray(fc_b, np.float32), 0.0)
    out = z @ np.asarray(out_W, np.float32) + np.asarray(out_b, np.float32)
    return out.astype(np.float32)


# revision 50
# speedup vs baseline: 1.0205x; 1.0205x over previous
"""Trainium2 Bass kernel for nn_CriticNetwork (gnn_message_passing).

Mathematical simplification (verified against the reference): the
reference broadcasts edge_index to (B, 2, E) and reshapes to
(2, B*E); row-major reshape makes src == dst elementwise, so every
edge is a self-edge and with GCN normalization both GCNConv layers
collapse to plain linear layers.  Since the post-relu node/col heads
are linear, the device only needs per-core SUMS of the hidden
activations; the host applies the tiny final heads.

v2 design (vs the 35.5us v1 baseline) — driven by NTFF trace analysis:
  * fp8(e4m3) x over the wire (1.6MB/core instead of 3.2MB bf16) and
    fp8 W1 with DoubleRow matmuls: contraction 2x128 packs FOUR
    64-feature nodes per moving column pair, halving L1 PE time.
    (W2/h1 stay bf16: measured node_avg rel-err 4e-4, budget 2e-2.)
  * Row-shaped output [8,128] fp32 via one fp32 PE transpose of the
    whole stats tile (host sums the chunk rows): the v1 [128,1]
    column output emitted 128 4-byte HBM descriptors ->
    read-modify-write grind, ~6us of tail.
  * Bulk x stream on ONE need-ordered HWDGE queue (parallel queues
    round-robin per packet and delay the first-needed transfer);
    small constants ride the scalar queue in parallel.  The DoubleRow
    stationaries ride the head of the first x transfer so the first
    matmul waits a single DMA round trip.
  * Minimal instruction count: measured ~255ns dispatch overhead per
    (dependent) instruction; v1 had 522 instructions.

Per-core layout (25000 nodes): 7 psum chunks (6 x 512 cols + 53).
Each psum column holds 8 nodes (128 rows = 8 nodes x 16 hidden).
Chunk x layout [128, 4, M]: blocks (A0,A1,B0,B1); DR matmul A
consumes blocks 0-1 -> psum rows 0:64, B -> rows 64:128.
x[p, hb, m] = feat (p%64) of node(chunk_base + (2*hb + p//64)*M + m).
"""

import ml_dtypes
import numpy as np

import concourse.bacc as bacc
import concourse.mybir as mybir
import concourse.tile as tile
from concourse.bass_utils import run_bass_kernel_spmd

P = 128
N_CORES = 8
B, N, F_NODE, H = 4, 50000, 64, 16
C, F_COL = 1000, 32
NODES_PER_CORE = (B * N) // N_CORES          # 25000
COLN = (B * C) // N_CORES                    # 500 col rows per core

MM = 512                                     # psum bank cols (fp32)
NODES_PER_CHUNK = 8 * MM                     # 4096
N_FULL = NODES_PER_CORE // NODES_PER_CHUNK   # 6 full chunks
# tail: 424 nodes -> 53 cols, padded to 64 (DoubleRow AP needs the
# k-pair stride %16 == 0 and an even column count); pad nodes are
# zero and the host subtracts their bias-path contribution.
M_TAIL = 64
N_PAD = 8 * M_TAIL - (NODES_PER_CORE - N_FULL * NODES_PER_CHUNK)    # 88
CHUNK_M = [MM] * N_FULL + [M_TAIL]           # 7 chunks
N_CHUNKS = len(CHUNK_M)

# wpack (bf16): just blockdiag(W2 x8) [128, 128]
NWP = P
# wcol (bf16): col_W1 [32, 0:16] + colT [32, 16:516] -- shipped late on
# the sync queue so the scheduler cannot hoist the col matmul ahead of
# the node chunks on the PE.
CW1_OFF = 0
COLT_OFF = H
NWC = COLT_OFF + COLN                        # 516

# bias32 (fp32, tiny, first on the scalar queue -- relu1 needs b1)
B1_OFF = 0                                   # [128, 1] b1 tiled x8
B2_OFF = 1                                   # [128, 1] b2 tiled x8
NB2_OFF = 2                                  # [128, 1] -b2 tiled x8
CB1_OFF = 3                                  # [16, 1]  col_b1
NB32 = 4

DT = mybir.dt.bfloat16
FP8 = mybir.dt.float8e4
NPBF = ml_dtypes.bfloat16
NPF8 = ml_dtypes.float8_e4m3                 # TRN FP8_EXP4-compatible
DR = mybir.MatmulPerfMode.DoubleRow

PROFILE = False
CHECK_WAITS = True
LAST_EXEC_TIME_NS = None
LAST_RESULTS = None

_NC_CACHE = {}


def _build_nc():
    f32 = mybir.dt.float32
    Relu = mybir.ActivationFunctionType.Relu
    nc = bacc.Bacc("TRN2")

    # Two full-partition DoubleRow stationaries (the ISA rejects
    # partition-sliced PSUM outputs in DR mode): w1A covers x blocks
    # 0-1 -> psum rows 0:64 (cols 64:128 zero), w1B covers blocks
    # 2-3 -> rows 64:128; the two matmuls accumulate into one bank.
    # w1A|w1B|chunk0|tail-chunk ride ONE first transfer (xw0): the first
    # compute waits a single DMA round trip, and the tiny tail chunk is
    # computed right after chunk 0 so the pipeline END gates on x3's
    # earlier semaphore instead of a trailing 32KB transfer.
    XW0W = 2 * P + 2 * MM + 2 * M_TAIL
    xw0 = nc.dram_tensor("xw0", [P, 2, XW0W], FP8, kind="ExternalInput")
    xt1 = nc.dram_tensor("xt1", [P, 8, MM], FP8, kind="ExternalInput")
    xt2 = nc.dram_tensor("xt2", [P, 8, MM], FP8, kind="ExternalInput")
    xt3 = nc.dram_tensor("xt3", [P, 4, MM], FP8, kind="ExternalInput")
    wpack = nc.dram_tensor("wpack", [P, NWP], DT, kind="ExternalInput")
    wcol = nc.dram_tensor("wcol", [F_COL, NWC], DT, kind="ExternalInput")
    bias32 = nc.dram_tensor("bias32", [P, NB32], f32, kind="ExternalInput")
    ident32 = nc.dram_tensor("ident32", [P, P], f32, kind="ExternalInput")
    out_acc = nc.dram_tensor("out_acc", [N_CHUNKS + 1, P], f32,
                             kind="ExternalOutput")

    with tile.TileContext(nc) as tc:
        with (
            tc.tile_pool(name="consts", bufs=1) as consts,
            tc.tile_pool(name="xin", bufs=1) as xin,
            tc.tile_pool(name="work", bufs=1) as work,
            tc.tile_pool(name="psum", bufs=1, space="PSUM") as psum,
        ):
            # --- input DMAs: ONE queue, strict need-order -------------
            # Parallel queues round-robin per packet, so they dilute
            # each other and the first-needed transfer lands almost as
            # late as the last.  A single FIFO queue completes in
            # consumption order at full bandwidth instead.
            # sync queue: the bulk x stream only, strict need-order (a
            # second bulk queue dilutes packet round-robin and delays
            # the first-needed transfer; constants ride scalar instead).
            x0 = xin.tile([P, 2, XW0W], FP8, tag="x0", name="x0")
            nc.sync.dma_start(x0[:, :, :], xw0[:, :, :])
            x1 = xin.tile([P, 8, MM], FP8, tag="x1", name="x1")
            nc.sync.dma_start(x1[:, :, :], xt1[:, :, :])
            x2 = xin.tile([P, 8, MM], FP8, tag="x2", name="x2")
            nc.sync.dma_start(x2[:, :, :], xt2[:, :, :])
            x3 = xin.tile([P, 4, MM], FP8, tag="x3", name="x3")
            nc.sync.dma_start(x3[:, :, :], xt3[:, :, :])
            bs = consts.tile([P, NB32], f32)
            nc.scalar.dma_start(bs[:], bias32[:])
            wp = consts.tile([P, NWP], DT)
            nc.scalar.dma_start(wp[:], wpack[:])
            # +1 spare col: DVE touches it mid-loop so the scheduler
            # cannot hoist the col matmul (which reads this tile) ahead
            # of the node chunks.
            wc = consts.tile([F_COL, NWC + 1], DT)
            nc.scalar.dma_start(wc[:, 0:NWC], wcol[:])
            nc.vector.memset(wc[:, NWC:NWC + 1], 0.0)
            i128t = consts.tile([P, P], f32)
            nc.scalar.dma_start(i128t[:], ident32[:])
            w1s = x0  # stationaries live in the head of xw0

            w2_t = wp[:, 0:P]
            cw1_t = wc[:, CW1_OFF:CW1_OFF + H]
            colT_t = wc[:, COLT_OFF:COLT_OFF + COLN]
            i128 = i128t[:, :]
            b1_t = bs[:, B1_OFF:B1_OFF + 1]
            b2_t = bs[:, B2_OFF:B2_OFF + 1]
            nb2_t = bs[:, NB2_OFF:NB2_OFF + 1]
            cb1_t = bs[:H, CB1_OFF:CB1_OFF + 1]

            # stats: col c = chunk-c row sums; col N_CHUNKS = col-path
            # totals (rows 0-15; rest zeroed).  The whole tile is
            # transposed out at the end; the host sums the chunk rows.
            stats = work.tile([P, N_CHUNKS + 1], f32)
            nc.vector.memset(stats[:, N_CHUNKS:N_CHUNKS + 1], 0.0)

            NBUF = 3
            ps1_t = [psum.tile([P, MM], f32, tag=f"ps1_{k}", name=f"ps1_{k}")
                     for k in range(NBUF)]
            ps2_t = [psum.tile([P, MM], f32, tag=f"ps2_{k}", name=f"ps2_{k}")
                     for k in range(NBUF)]
            h1_t = [work.tile([P, MM], DT, tag=f"h1_{k}", name=f"h1_{k}")
                    for k in range(NBUF)]
            scr_t = [work.tile([P, MM], DT, tag=f"scr_{k}", name=f"scr_{k}")
                     for k in range(NBUF)]

            # emission order: chunk0, tail chunk (both ride xw0), then
            # the stream chunks; -> (rhs A slice, rhs B slice, M)
            W0 = 2 * P             # chunk0 data offset inside xw0
            WT = W0 + 2 * MM       # tail chunk offset inside xw0
            srcs = [
                (x0[:, 0:2, W0:W0 + MM],
                 x0[:, 0:2, W0 + MM:W0 + 2 * MM], MM),
                (x0[:, 0:2, WT:WT + M_TAIL],
                 x0[:, 0:2, WT + M_TAIL:WT + 2 * M_TAIL], M_TAIL),
                (x1[:, 0:2, :], x1[:, 2:4, :], MM),
                (x1[:, 4:6, :], x1[:, 6:8, :], MM),
                (x2[:, 0:2, :], x2[:, 2:4, :], MM),
                (x2[:, 4:6, :], x2[:, 6:8, :], MM),
                (x3[:, 0:2, :], x3[:, 2:4, :], MM),
            ]

            for c, (rhsA, rhsB, M) in enumerate(srcs):
                ps1 = ps1_t[c % NBUF]
                nc.tensor.matmul(
                    ps1[:, :M], w1s[:, :, 0:P], rhsA,
                    start=True, stop=False, perf_mode=DR)
                nc.tensor.matmul(
                    ps1[:, :M], w1s[:, :, P:2 * P], rhsB,
                    start=False, stop=True, perf_mode=DR)
                h1 = h1_t[c % NBUF]
                nc.scalar.activation(h1[:, :M], ps1[:, :M], Relu, bias=b1_t)
                ps2 = ps2_t[c % NBUF]
                nc.tensor.matmul(ps2[:, :M], w2_t, h1[:, :M],
                                 start=True, stop=True)
                scr = scr_t[c % NBUF]
                # relu(x + b2) = max(x, -b2) + b2 -- the DVE's second ALU
                # stage does not apply `max`, so keep max in stage 0.
                nc.vector.tensor_scalar(
                    scr[:, :M], ps2[:, :M], nb2_t, b2_t,
                    mybir.AluOpType.max, mybir.AluOpType.add,
                    accum_out=stats[:, c:c + 1])
                if c == 2:
                    # pin the col matmul behind chunk 2: write the spare
                    # col its rhs covers (junk col excluded from accum).
                    nc.vector.tensor_copy(wc[0:1, NWC:NWC + 1],
                                          stats[0:1, 2:3])

            # column-features path: h = relu(colT.T @ col_W1 + col_b1).
            # rhs covers one junk col (the scheduling pin); the accum
            # reads only the first COLN psum columns.
            psc = psum.tile([H, COLN + 1], f32, tag="psc")
            nc.tensor.matmul(psc[:, :], cw1_t,
                             wc[:, COLT_OFF:COLT_OFF + COLN + 1],
                             start=True, stop=True)
            colscr = work.tile([H, COLN], f32)
            nc.scalar.activation(colscr[:], psc[:, 0:COLN], Relu, bias=cb1_t,
                                 accum_out=stats[:H, N_CHUNKS:N_CHUNKS + 1])

            # transpose all stats columns -> [8, 128] rows on the PE
            ptr = psum.tile([N_CHUNKS + 1, P], f32, tag="ptr")
            nc.tensor.transpose(ptr[:, :], stats[:, :], i128)
            row = work.tile([N_CHUNKS + 1, P], f32)
            nc.vector.tensor_copy(row[:], ptr[:])
            nc.sync.dma_start(out_acc[:], row[:])

    nc.finalize()

    if CHECK_WAITS:
        for blk in nc.m.functions[0].blocks:
            for inst in blk.instructions:
                si = inst.sync_info
                nwait = len(si.on_wait) if si and si.on_wait else 0
                limit = 2 if type(inst).__name__ in (
                    "InstEventSemaphore", "InstDrain", "InstDMACopy") else 1
                assert nwait <= limit, (
                    inst.name, type(inst).__name__,
                    [w.ant_name for w in si.on_wait])
    return nc


def _get_nc():
    if "nc" not in _NC_CACHE:
        _NC_CACHE["nc"] = _build_nc()
    return _NC_CACHE["nc"]


def _pack_x_core(xc):
    """xc [25000, 64] f32 -> [128, 12544] fp8 in chunked DR layout."""
    if N_PAD:
        xc = np.concatenate(
            [xc, np.zeros((N_PAD, F_NODE), xc.dtype)], axis=0)
    cols = []
    base = 0
    for M in CHUNK_M:
        nodes = xc[base:base + 8 * M]                 # [8M, 64]
        a = nodes.reshape(4, 2, M, F_NODE)            # (hb, prow, m, feat)
        cols.append(a.transpose(1, 3, 0, 2).reshape(P, 4 * M))
        base += 8 * M
    return np.concatenate(cols, axis=1)


def _prep_in_maps(node_features, col_features, W1, b1, W2, b2, col_W1, col_b1):
    f32 = np.float32
    x = np.ascontiguousarray(node_features, dtype=f32).reshape(B * N, F_NODE)
    colf = np.ascontiguousarray(col_features, dtype=f32).reshape(B * C, F_COL)
    W1 = np.asarray(W1, f32)
    W2 = np.asarray(W2, f32)

    # DoubleRow stationaries: w1X[p, k, 16q+f] = W1[p%64, f] where
    # q = 2*k + p//64 (4 nodes per moving column pair); w1A fills
    # out rows 0:64 (blocks 0-1), w1B rows 64:128 (blocks 2-3).
    w1dr = np.zeros((P, 2, 2 * P), f32)
    for k in range(2):
        for ph in range(2):
            q = 2 * k + ph
            w1dr[ph * 64:(ph + 1) * 64, k, 16 * q:16 * q + H] = W1          # A
            w1dr[ph * 64:(ph + 1) * 64, k, P + 64 + 16 * q:P + 64 + 16 * q + H] = W1  # B
    w1dr = w1dr.astype(NPF8)

    wpack = np.zeros((P, NWP), f32)
    for g in range(P // H):
        wpack[H * g:H * g + H, H * g:H * g + H] = W2
    wpack = wpack.astype(NPBF)

    wcol_base = np.zeros((F_COL, NWC), f32)
    wcol_base[:, CW1_OFF:CW1_OFF + H] = np.asarray(col_W1, f32)

    bias32 = np.zeros((P, NB32), f32)
    bias32[:, B1_OFF] = np.tile(np.asarray(b1, f32), P // H)
    bias32[:, B2_OFF] = np.tile(np.asarray(b2, f32), P // H)
    bias32[:, NB2_OFF] = -bias32[:, B2_OFF]
    bias32[:H, CB1_OFF] = np.asarray(col_b1, f32)
    ident32 = np.eye(P, dtype=f32)

    bounds = np.cumsum([0] + [4 * M for M in CHUNK_M])  # chunk col offsets

    in_maps = []
    for core in range(N_CORES):
        n0 = core * NODES_PER_CORE
        xp = _pack_x_core(x[n0:n0 + NODES_PER_CORE]).astype(NPF8)
        wcol = wcol_base.copy()
        wcol[:, COLT_OFF:COLT_OFF + COLN] = \
            colf[core * COLN:(core + 1) * COLN].T
        # xw0 [128, 2, 256+1024+128]: w1A | w1B | chunk0-A | chunk0-B |
        # tail-A | tail-B; dim1 is the DoubleRow k index.
        c0 = xp[:, bounds[0]:bounds[1]].reshape(P, 4, MM)
        ct = xp[:, bounds[6]:bounds[7]].reshape(P, 4, M_TAIL)
        xw0 = np.concatenate(
            [w1dr, c0[:, 0:2, :], c0[:, 2:4, :],
             ct[:, 0:2, :], ct[:, 2:4, :]], axis=2)
        in_maps.append({
            "xw0": np.ascontiguousarray(xw0),
            "xt1": xp[:, bounds[1]:bounds[3]].reshape(P, 8, MM),
            "xt2": xp[:, bounds[3]:bounds[5]].reshape(P, 8, MM),
            "xt3": xp[:, bounds[5]:bounds[6]].reshape(P, 4, MM),
            "wpack": wpack,
            "wcol": wcol.astype(NPBF),
            "bias32": bias32,
            "ident32": ident32,
        })
    return in_maps


def kernel(node_features, col_features, edge_index, W1, b1, W2, b2,
           node_fc_W, node_fc_b, col_W1, col_b1, col_W2, col_b2,
           fc_W, fc_b, out_W, out_b):
    global LAST_EXEC_TIME_NS, LAST_RESULTS
    # edge_index provably does not affect the output (see module docstring).
    in_maps = _prep_in_maps(node_features, col_features,
                            W1, b1, W2, b2, col_W1, col_b1)
    nc = _get_nc()
    res = run_bass_kernel_spmd(nc, in_maps, core_ids=list(range(N_CORES)),
                               trace=PROFILE)
    LAST_EXEC_TIME_NS = res.exec_time_ns
    LAST_RESULTS = res
    outs = res.results

    node_fc_W = np.asarray(node_fc_W, np.float32)
    col_W2 = np.asarray(col_W2, np.float32)
    # Device relu2 computes max(x, -b2) (+b2 once per chunk-reduction),
    # i.e. each chunk col = sum(relu) - (M-1)*b2; add the constant back.
    # Zero-pad nodes contribute relu(W2.T relu(b1) + b2) each; subtract.
    b2f = np.asarray(b2, np.float32)
    summ1 = np.float32(sum(M - 1 for M in CHUNK_M))
    pad_h2 = np.maximum(
        np.maximum(np.asarray(b1, np.float32), 0.0) @ np.asarray(W2, np.float32)
        + b2f, 0.0) * np.float32(N_PAD) - summ1 * b2f * np.float32(P // H)
    node_avg = np.zeros((B, 1), np.float32)
    col_avg = np.zeros((B, 1), np.float32)
    for b in range(B):
        acc = (outs[2 * b]["out_acc"].astype(np.float32) +
               outs[2 * b + 1]["out_acc"].astype(np.float32))
        ns = acc[:N_CHUNKS].sum(axis=0).reshape(P // H, H).sum(axis=0) \
            - 2.0 * pad_h2
        cs = acc[N_CHUNKS, :H]                           # col feature sums
        node_avg[b, 0] = (ns / np.float32(N)) @ node_fc_W[:, 0] + \
            np.asarray(node_fc_b, np.float32)[0]
        col_avg[b, 0] = (cs / np.float32(C)) @ col_W2[:, 0] + \
            np.asarray(col_b2, np.float32)[0]

    combined = np.concatenate([node_avg, col_avg], axis=1)      # [B, 2]
    z = np.maximum(combined @ np.asarray(fc_W, np.float32) +
                   np.asarray(fc_b, np.float32), 0.0)
    out = z @ np.asarray(out_W, np.float32) + np.asarray(out_b, np.float32)
    return out.astype(np.float32)


# revision 60
# speedup vs baseline: 1.0302x; 1.0095x over previous
"""Trainium2 Bass kernel for nn_CriticNetwork (gnn_message_passing).

Mathematical simplification (verified against the reference): the
reference broadcasts edge_index to (B, 2, E) and reshapes to
(2, B*E); row-major reshape makes src == dst elementwise, so every
edge is a self-edge and with GCN normalization both GCNConv layers
collapse to plain linear layers.  Since the post-relu node/col heads
are linear, the device only needs per-core SUMS of the hidden
activations; the host applies the tiny final heads.

v2 design (vs the 35.5us v1 baseline) — driven by NTFF trace analysis:
  * fp8(e4m3) x over the wire (1.6MB/core instead of 3.2MB bf16) and
    fp8 W1 with DoubleRow matmuls: contraction 2x128 packs FOUR
    64-feature nodes per moving column pair, halving L1 PE time.
    (W2/h1 stay bf16: measured node_avg rel-err 4e-4, budget 2e-2.)
  * Row-shaped output [8,128] fp32 via one fp32 PE transpose of the
    whole stats tile (host sums the chunk rows): the v1 [128,1]
    column output emitted 128 4-byte HBM descriptors ->
    read-modify-write grind, ~6us of tail.
  * Bulk x stream on ONE need-ordered HWDGE queue (parallel queues
    round-robin per packet and delay the first-needed transfer);
    small constants ride the scalar queue in parallel.  The DoubleRow
    stationaries ride the head of the first x transfer so the first
    matmul waits a single DMA round trip.
  * Minimal instruction count: measured ~255ns dispatch overhead per
    (dependent) instruction; v1 had 522 instructions.

Per-core layout (25000 nodes): 7 psum chunks (6 x 512 cols + 53).
Each psum column holds 8 nodes (128 rows = 8 nodes x 16 hidden).
Chunk x layout [128, 4, M]: blocks (A0,A1,B0,B1); DR matmul A
consumes blocks 0-1 -> psum rows 0:64, B -> rows 64:128.
x[p, hb, m] = feat (p%64) of node(chunk_base + (2*hb + p//64)*M + m).
"""

import ml_dtypes
import numpy as np

import concourse.bacc as bacc
import concourse.mybir as mybir
import concourse.tile as tile
from concourse.bass_utils import run_bass_kernel_spmd

P = 128
N_CORES = 8
B, N, F_NODE, H = 4, 50000, 64, 16
C, F_COL = 1000, 32
NODES_PER_CORE = (B * N) // N_CORES          # 25000
COLN = (B * C) // N_CORES                    # 500 col rows per core

MM = 512                                     # psum bank cols (fp32)
NODES_PER_CHUNK = 8 * MM                     # 4096
N_FULL = NODES_PER_CORE // NODES_PER_CHUNK   # 6 full chunks
# tail: 424 nodes -> 53 cols, padded to 64 (DoubleRow AP needs the
# k-pair stride %16 == 0 and an even column count); pad nodes are
# zero and the host subtracts their bias-path contribution.
M_TAIL = 64
N_PAD = 8 * M_TAIL - (NODES_PER_CORE - N_FULL * NODES_PER_CHUNK)    # 88
CHUNK_M = [MM] * N_FULL + [M_TAIL]           # 7 chunks
N_CHUNKS = len(CHUNK_M)

# wpack (bf16): just blockdiag(W2 x8) [128, 128]
NWP = P
# wcol (bf16): col_W1 [32, 0:16] + colT [32, 16:516] -- shipped late on
# the sync queue so the scheduler cannot hoist the col matmul ahead of
# the node chunks on the PE.
CW1_OFF = 0
COLT_OFF = H
NWC = COLT_OFF + COLN                        # 516

# bias32 (fp32, tiny, first on the scalar queue -- relu1 needs b1)
B1_OFF = 0                                   # [128, 1] b1 tiled x8
B2_OFF = 1                                   # [128, 1] b2 tiled x8
NB2_OFF = 2                                  # [128, 1] -b2 tiled x8
CB1_OFF = 3                                  # [16, 1]  col_b1
NB32 = 4

DT = mybir.dt.bfloat16
FP8 = mybir.dt.float8e4
NPBF = ml_dtypes.bfloat16
NPF8 = ml_dtypes.float8_e4m3                 # TRN FP8_EXP4-compatible
DR = mybir.MatmulPerfMode.DoubleRow

PROFILE = False
CHECK_WAITS = True
LAST_EXEC_TIME_NS = None
LAST_RESULTS = None

_NC_CACHE = {}


def _build_nc():
    f32 = mybir.dt.float32
    Relu = mybir.ActivationFunctionType.Relu
    nc = bacc.Bacc("TRN2")

    # Two full-partition DoubleRow stationaries (the ISA rejects
    # partition-sliced PSUM outputs in DR mode): w1A covers x blocks
    # 0-1 -> psum rows 0:64 (cols 64:128 zero), w1B covers blocks
    # 2-3 -> rows 64:128; the two matmuls accumulate into one bank.
    # w1A|w1B|chunk0|tail-chunk ride ONE first transfer (xw0): the first
    # compute waits a single DMA round trip, and the tiny tail chunk is
    # computed right after chunk 0 so the pipeline END gates on x3's
    # earlier semaphore instead of a trailing 32KB transfer.
    XW0W = 2 * P + 2 * MM + 2 * M_TAIL
    xw0 = nc.dram_tensor("xw0", [P, 2, XW0W], FP8, kind="ExternalInput")
    xt1 = nc.dram_tensor("xt1", [P, 8, MM], FP8, kind="ExternalInput")
    xt2 = nc.dram_tensor("xt2", [P, 8, MM], FP8, kind="ExternalInput")
    xt3 = nc.dram_tensor("xt3", [P, 4, MM], FP8, kind="ExternalInput")
    wpack = nc.dram_tensor("wpack", [P, NWP], DT, kind="ExternalInput")
    wcol = nc.dram_tensor("wcol", [F_COL, NWC], DT, kind="ExternalInput")
    bias32 = nc.dram_tensor("bias32", [P, NB32], f32, kind="ExternalInput")
    ident32 = nc.dram_tensor("ident32", [P, P], f32, kind="ExternalInput")
    out_a = nc.dram_tensor("out_a", [N_CHUNKS, P], f32,
                           kind="ExternalOutput")
    out_b = nc.dram_tensor("out_b", [1, P], f32, kind="ExternalOutput")

    with tile.TileContext(nc) as tc:
        with (
            tc.tile_pool(name="consts", bufs=1) as consts,
            tc.tile_pool(name="xin", bufs=1) as xin,
            tc.tile_pool(name="work", bufs=1) as work,
            tc.tile_pool(name="psum", bufs=1, space="PSUM") as psum,
        ):
            # --- input DMAs: ONE queue, strict need-order -------------
            # Parallel queues round-robin per packet, so they dilute
            # each other and the first-needed transfer lands almost as
            # late as the last.  A single FIFO queue completes in
            # consumption order at full bandwidth instead.
            # sync queue: the bulk x stream only, strict need-order (a
            # second bulk queue dilutes packet round-robin and delays
            # the first-needed transfer; constants ride scalar instead).
            x0 = xin.tile([P, 2, XW0W], FP8, tag="x0", name="x0")
            nc.sync.dma_start(x0[:, :, :], xw0[:, :, :])
            x1 = xin.tile([P, 8, MM], FP8, tag="x1", name="x1")
            nc.sync.dma_start(x1[:, :, :], xt1[:, :, :])
            x2 = xin.tile([P, 8, MM], FP8, tag="x2", name="x2")
            nc.sync.dma_start(x2[:, :, :], xt2[:, :, :])
            x3 = xin.tile([P, 4, MM], FP8, tag="x3", name="x3")
            nc.sync.dma_start(x3[:, :, :], xt3[:, :, :])
            bs = consts.tile([P, NB32], f32)
            nc.scalar.dma_start(bs[:], bias32[:])
            wp = consts.tile([P, NWP], DT)
            nc.scalar.dma_start(wp[:], wpack[:])
            # +1 spare col: DVE touches it mid-loop so the scheduler
            # cannot hoist the col matmul (which reads this tile) ahead
            # of the node chunks.
            wc = consts.tile([F_COL, NWC + 1], DT)
            nc.scalar.dma_start(wc[:, 0:NWC], wcol[:])
            nc.vector.memset(wc[:, NWC:NWC + 1], 0.0)
            i128t = consts.tile([P, P], f32)
            nc.scalar.dma_start(i128t[:], ident32[:])
            w1s = x0  # stationaries live in the head of xw0

            w2_t = wp[:, 0:P]
            cw1_t = wc[:, CW1_OFF:CW1_OFF + H]
            colT_t = wc[:, COLT_OFF:COLT_OFF + COLN]
            i128 = i128t[:, :]
            b1_t = bs[:, B1_OFF:B1_OFF + 1]
            b2_t = bs[:, B2_OFF:B2_OFF + 1]
            nb2_t = bs[:, NB2_OFF:NB2_OFF + 1]
            cb1_t = bs[:H, CB1_OFF:CB1_OFF + 1]

            # stats: cols 0:6 = first six chunks, col 6 = col-path
            # (rows 0-15; rest zeroed), col 7 = LAST chunk.  Cols 0:7
            # transpose early (hidden under last-chunk compute); only
            # col 7 transposes on the critical tail.
            stats = work.tile([P, N_CHUNKS + 1], f32)
            nc.vector.memset(stats[:, N_CHUNKS - 1:N_CHUNKS], 0.0)

            NBUF = 3
            ps1_t = [psum.tile([P, MM], f32, tag=f"ps1_{k}", name=f"ps1_{k}")
                     for k in range(NBUF)]
            ps2_t = [psum.tile([P, MM], f32, tag=f"ps2_{k}", name=f"ps2_{k}")
                     for k in range(2)]
            h1_t = [work.tile([P, MM], DT, tag=f"h1_{k}", name=f"h1_{k}")
                    for k in range(NBUF)]
            scr_t = [work.tile([P, MM], DT, tag=f"scr_{k}", name=f"scr_{k}")
                     for k in range(NBUF)]

            # emission order: chunk0, tail chunk (both ride xw0), then
            # the stream chunks; -> (rhs A slice, rhs B slice, M)
            W0 = 2 * P             # chunk0 data offset inside xw0
            WT = W0 + 2 * MM       # tail chunk offset inside xw0
            srcs = [
                (x0[:, 0:2, W0:W0 + MM],
                 x0[:, 0:2, W0 + MM:W0 + 2 * MM], MM),
                (x0[:, 0:2, WT:WT + M_TAIL],
                 x0[:, 0:2, WT + M_TAIL:WT + 2 * M_TAIL], M_TAIL),
                (x1[:, 0:2, :], x1[:, 2:4, :], MM),
                (x1[:, 4:6, :], x1[:, 6:8, :], MM),
                (x2[:, 0:2, :], x2[:, 2:4, :], MM),
                (x2[:, 4:6, :], x2[:, 6:8, :], MM),
                (x3[:, 0:2, :], x3[:, 2:4, :], MM),
            ]

            for c, (rhsA, rhsB, M) in enumerate(srcs):
                ps1 = ps1_t[c % NBUF]
                nc.tensor.matmul(
                    ps1[:, :M], w1s[:, :, 0:P], rhsA,
                    start=True, stop=False, perf_mode=DR)
                nc.tensor.matmul(
                    ps1[:, :M], w1s[:, :, P:2 * P], rhsB,
                    start=False, stop=True, perf_mode=DR)
                h1 = h1_t[c % NBUF]
                nc.scalar.activation(h1[:, :M], ps1[:, :M], Relu, bias=b1_t)
                ps2 = ps2_t[c % 2]
                nc.tensor.matmul(ps2[:, :M], w2_t, h1[:, :M],
                                 start=True, stop=True)
                scr = scr_t[c % NBUF]
                # relu(x + b2) = max(x, -b2) + b2 -- the DVE's second ALU
                # stage does not apply `max`, so keep max in stage 0.
                sc = c if c < len(srcs) - 1 else N_CHUNKS  # last -> col 7
                nc.vector.tensor_scalar(
                    scr[:, :M], ps2[:, :M], nb2_t, b2_t,
                    mybir.AluOpType.max, mybir.AluOpType.add,
                    accum_out=stats[:, sc:sc + 1])
                if c == 2:
                    # pin the col matmul behind chunk 2: write the spare
                    # col its rhs covers (junk col excluded from accum).
                    nc.vector.tensor_copy(wc[0:1, NWC:NWC + 1],
                                          stats[0:1, 2:3])

            # column-features path: h = relu(colT.T @ col_W1 + col_b1).
            # rhs covers one junk col (the scheduling pin); the accum
            # reads only the first COLN psum columns.
            psc = psum.tile([H, COLN + 1], f32, tag="psc")
            nc.tensor.matmul(psc[:, :], cw1_t,
                             wc[:, COLT_OFF:COLT_OFF + COLN + 1],
                             start=True, stop=True)
            colscr = work.tile([H, COLN], f32)
            nc.scalar.activation(colscr[:], psc[:, 0:COLN], Relu, bias=cb1_t,
                                 accum_out=stats[:H, N_CHUNKS - 1:N_CHUNKS])

            # transpose + ship cols 0:7 early (overlapping last-chunk
            # compute, including the out_a DMA receipt); only col 7
            # (the last chunk) rides the critical tail, via a second
            # transpose at psum partition 32 (dst must be 32-aligned).
            ptr = psum.tile([N_CHUNKS, P], f32, tag="ptr")
            ptr2 = psum.tile([1, P], f32, tag="ptr2")
            rowa = work.tile([N_CHUNKS, P], f32)
            rowb = work.tile([1, P], f32)
            nc.tensor.transpose(ptr[:, :], stats[:, 0:N_CHUNKS], i128)
            nc.vector.tensor_copy(rowa[:, :], ptr[:, :])
            nc.sync.dma_start(out_a[:], rowa[:])
            nc.tensor.transpose(ptr2[:, :],
                                stats[:, N_CHUNKS:N_CHUNKS + 1], i128)
            nc.vector.tensor_copy(rowb[:, :], ptr2[:, :])
            nc.sync.dma_start(out_b[:], rowb[:])

    nc.finalize()

    if CHECK_WAITS:
        for blk in nc.m.functions[0].blocks:
            for inst in blk.instructions:
                si = inst.sync_info
                nwait = len(si.on_wait) if si and si.on_wait else 0
                limit = 2 if type(inst).__name__ in (
                    "InstEventSemaphore", "InstDrain", "InstDMACopy") else 1
                assert nwait <= limit, (
                    inst.name, type(inst).__name__,
                    [w.ant_name for w in si.on_wait])
    return nc


def _get_nc():
    if "nc" not in _NC_CACHE:
        _NC_CACHE["nc"] = _build_nc()
    return _NC_CACHE["nc"]


def _pack_x_core(xc):
    """xc [25000, 64] f32 -> [128, 12544] fp8 in chunked DR layout."""
    if N_PAD:
        xc = np.concatenate(
            [xc, np.zeros((N_PAD, F_NODE), xc.dtype)], axis=0)
    cols = []
    base = 0
    for M in CHUNK_M:
        nodes = xc[base:base + 8 * M]                 # [8M, 64]
        a = nodes.reshape(4, 2, M, F_NODE)            # (hb, prow, m, feat)
        cols.append(a.transpose(1, 3, 0, 2).reshape(P, 4 * M))
        base += 8 * M
    return np.concatenate(cols, axis=1)


def _prep_in_maps(node_features, col_features, W1, b1, W2, b2, col_W1, col_b1):
    f32 = np.float32
    x = np.ascontiguousarray(node_features, dtype=f32).reshape(B * N, F_NODE)
    colf = np.ascontiguousarray(col_features, dtype=f32).reshape(B * C, F_COL)
    W1 = np.asarray(W1, f32)
    W2 = np.asarray(W2, f32)

    # DoubleRow stationaries: w1X[p, k, 16q+f] = W1[p%64, f] where
    # q = 2*k + p//64 (4 nodes per moving column pair); w1A fills
    # out rows 0:64 (blocks 0-1), w1B rows 64:128 (blocks 2-3).
    w1dr = np.zeros((P, 2, 2 * P), f32)
    for k in range(2):
        for ph in range(2):
            q = 2 * k + ph
            w1dr[ph * 64:(ph + 1) * 64, k, 16 * q:16 * q + H] = W1          # A
            w1dr[ph * 64:(ph + 1) * 64, k, P + 64 + 16 * q:P + 64 + 16 * q + H] = W1  # B
    w1dr = w1dr.astype(NPF8)

    wpack = np.zeros((P, NWP), f32)
    for g in range(P // H):
        wpack[H * g:H * g + H, H * g:H * g + H] = W2
    wpack = wpack.astype(NPBF)

    wcol_base = np.zeros((F_COL, NWC), f32)
    wcol_base[:, CW1_OFF:CW1_OFF + H] = np.asarray(col_W1, f32)

    bias32 = np.zeros((P, NB32), f32)
    bias32[:, B1_OFF] = np.tile(np.asarray(b1, f32), P // H)
    bias32[:, B2_OFF] = np.tile(np.asarray(b2, f32), P // H)
    bias32[:, NB2_OFF] = -bias32[:, B2_OFF]
    bias32[:H, CB1_OFF] = np.asarray(col_b1, f32)
    ident32 = np.eye(P, dtype=f32)

    bounds = np.cumsum([0] + [4 * M for M in CHUNK_M])  # chunk col offsets

    in_maps = []
    for core in range(N_CORES):
        n0 = core * NODES_PER_CORE
        xp = _pack_x_core(x[n0:n0 + NODES_PER_CORE]).astype(NPF8)
        wcol = wcol_base.copy()
        wcol[:, COLT_OFF:COLT_OFF + COLN] = \
            colf[core * COLN:(core + 1) * COLN].T
        # xw0 [128, 2, 256+1024+128]: w1A | w1B | chunk0-A | chunk0-B |
        # tail-A | tail-B; dim1 is the DoubleRow k index.
        c0 = xp[:, bounds[0]:bounds[1]].reshape(P, 4, MM)
        ct = xp[:, bounds[6]:bounds[7]].reshape(P, 4, M_TAIL)
        xw0 = np.concatenate(
            [w1dr, c0[:, 0:2, :], c0[:, 2:4, :],
             ct[:, 0:2, :], ct[:, 2:4, :]], axis=2)
        in_maps.append({
            "xw0": np.ascontiguousarray(xw0),
            "xt1": xp[:, bounds[1]:bounds[3]].reshape(P, 8, MM),
            "xt2": xp[:, bounds[3]:bounds[5]].reshape(P, 8, MM),
            "xt3": xp[:, bounds[5]:bounds[6]].reshape(P, 4, MM),
            "wpack": wpack,
            "wcol": wcol.astype(NPBF),
            "bias32": bias32,
            "ident32": ident32,
        })
    return in_maps


def kernel(node_features, col_features, edge_index, W1, b1, W2, b2,
           node_fc_W, node_fc_b, col_W1, col_b1, col_W2, col_b2,
           fc_W, fc_b, out_W, out_b):
    global LAST_EXEC_TIME_NS, LAST_RESULTS
    # edge_index provably does not affect the output (see module docstring).
    in_maps = _prep_in_maps(node_features, col_features,
                            W1, b1, W2, b2, col_W1, col_b1)
    nc = _get_nc()
    res = run_bass_kernel_spmd(nc, in_maps, core_ids=list(range(N_CORES)),
                               trace=PROFILE)
    LAST_EXEC_TIME_NS = res.exec_time_ns
    LAST_RESULTS = res
    outs = res.results

    node_fc_W = np.asarray(node_fc_W, np.float32)
    col_W2 = np.asarray(col_W2, np.float32)
    # Device relu2 computes max(x, -b2) (+b2 once per chunk-reduction),
    # i.e. each chunk col = sum(relu) - (M-1)*b2; add the constant back.
    # Zero-pad nodes contribute relu(W2.T relu(b1) + b2) each; subtract.
    b2f = np.asarray(b2, np.float32)
    summ1 = np.float32(sum(M - 1 for M in CHUNK_M))
    pad_h2 = np.maximum(
        np.maximum(np.asarray(b1, np.float32), 0.0) @ np.asarray(W2, np.float32)
        + b2f, 0.0) * np.float32(N_PAD) - summ1 * b2f * np.float32(P // H)
    node_avg = np.zeros((B, 1), np.float32)
    col_avg = np.zeros((B, 1), np.float32)
    for b in range(B):
        acc = (outs[2 * b]["out_a"].astype(np.float32) +
               outs[2 * b + 1]["out_a"].astype(np.float32))
        accb = (outs[2 * b]["out_b"].astype(np.float32) +
                outs[2 * b + 1]["out_b"].astype(np.float32))
        node_rows = acc[:N_CHUNKS - 1].sum(axis=0) + accb[0]
        ns = node_rows.reshape(P // H, H).sum(axis=0) - 2.0 * pad_h2
        cs = acc[N_CHUNKS - 1, :H]                       # col feature sums
        node_avg[b, 0] = (ns / np.float32(N)) @ node_fc_W[:, 0] + \
            np.asarray(node_fc_b, np.float32)[0]
        col_avg[b, 0] = (cs / np.float32(C)) @ col_W2[:, 0] + \
            np.asarray(col_b2, np.float32)[0]

    combined = np.concatenate([node_avg, col_avg], axis=1)      # [B, 2]
    z = np.maximum(combined @ np.asarray(fc_W, np.float32) +
                   np.asarray(fc_b, np.float32), 0.0)
    out = z @ np.asarray(out_W, np.float32) + np.asarray(out_b, np.float32)
    return out.astype(np.float32)


# revision 61
# speedup vs baseline: 1.0524x; 1.0216x over previous
"""Trainium2 Bass kernel for nn_CriticNetwork (gnn_message_passing).

Mathematical simplification (verified against the reference): the
reference broadcasts edge_index to (B, 2, E) and reshapes to
(2, B*E); row-major reshape makes src == dst elementwise, so every
edge is a self-edge and with GCN normalization both GCNConv layers
collapse to plain linear layers.  Since the post-relu node/col heads
are linear, the device only needs per-core SUMS of the hidden
activations; the host applies the tiny final heads.

v2 design (vs the 35.5us v1 baseline) — driven by NTFF trace analysis:
  * fp8(e4m3) x over the wire (1.6MB/core instead of 3.2MB bf16) and
    fp8 W1 with DoubleRow matmuls: contraction 2x128 packs FOUR
    64-feature nodes per moving column pair, halving L1 PE time.
    (W2/h1 stay bf16: measured node_avg rel-err 4e-4, budget 2e-2.)
  * Row-shaped output [8,128] fp32 via one fp32 PE transpose of the
    whole stats tile (host sums the chunk rows): the v1 [128,1]
    column output emitted 128 4-byte HBM descriptors ->
    read-modify-write grind, ~6us of tail.
  * Bulk x stream on ONE need-ordered HWDGE queue (parallel queues
    round-robin per packet and delay the first-needed transfer);
    small constants ride the scalar queue in parallel.  The DoubleRow
    stationaries ride the head of the first x transfer so the first
    matmul waits a single DMA round trip.
  * Minimal instruction count: measured ~255ns dispatch overhead per
    (dependent) instruction; v1 had 522 instructions.

Per-core layout (25000 nodes): 7 psum chunks (6 x 512 cols + 53).
Each psum column holds 8 nodes (128 rows = 8 nodes x 16 hidden).
Chunk x layout [128, 4, M]: blocks (A0,A1,B0,B1); DR matmul A
consumes blocks 0-1 -> psum rows 0:64, B -> rows 64:128.
x[p, hb, m] = feat (p%64) of node(chunk_base + (2*hb + p//64)*M + m).
"""

import ml_dtypes
import numpy as np

import concourse.bacc as bacc
import concourse.mybir as mybir
import concourse.tile as tile
from concourse.bass_utils import run_bass_kernel_spmd

P = 128
N_CORES = 8
B, N, F_NODE, H = 4, 50000, 64, 16
C, F_COL = 1000, 32
NODES_PER_CORE = (B * N) // N_CORES          # 25000
COLN = (B * C) // N_CORES                    # 500 col rows per core

MM = 512                                     # psum bank cols (fp32)
NODES_PER_CHUNK = 8 * MM                     # 4096
N_FULL = NODES_PER_CORE // NODES_PER_CHUNK   # 6 full chunks
# tail: 424 nodes -> 53 cols, padded to 64 (DoubleRow AP needs the
# k-pair stride %16 == 0 and an even column count); pad nodes are
# zero and the host subtracts their bias-path contribution.
M_TAIL = 64
N_PAD = 8 * M_TAIL - (NODES_PER_CORE - N_FULL * NODES_PER_CHUNK)    # 88
CHUNK_M = [MM] * N_FULL + [M_TAIL]           # 7 chunks
N_CHUNKS = len(CHUNK_M)

# wpack (bf16): just blockdiag(W2 x8) [128, 128]
NWP = P
# wcol (bf16): col_W1 [32, 0:16] + colT [32, 16:516] -- shipped late on
# the sync queue so the scheduler cannot hoist the col matmul ahead of
# the node chunks on the PE.
CW1_OFF = 0
COLT_OFF = H
NWC = COLT_OFF + COLN                        # 516

# bias32 (fp32, tiny, first on the scalar queue -- relu1 needs b1)
B1_OFF = 0                                   # [128, 1] b1 tiled x8
B2_OFF = 1                                   # [128, 1] b2 tiled x8
NB2_OFF = 2                                  # [128, 1] -b2 tiled x8
CB1_OFF = 3                                  # [16, 1]  col_b1
NB32 = 4

DT = mybir.dt.bfloat16
FP8 = mybir.dt.float8e4
NPBF = ml_dtypes.bfloat16
NPF8 = ml_dtypes.float8_e4m3                 # TRN FP8_EXP4-compatible
DR = mybir.MatmulPerfMode.DoubleRow

PROFILE = False
CHECK_WAITS = True
LAST_EXEC_TIME_NS = None
LAST_RESULTS = None

_NC_CACHE = {}


def _build_nc():
    f32 = mybir.dt.float32
    Relu = mybir.ActivationFunctionType.Relu
    nc = bacc.Bacc("TRN2")

    # Two full-partition DoubleRow stationaries (the ISA rejects
    # partition-sliced PSUM outputs in DR mode): w1A covers x blocks
    # 0-1 -> psum rows 0:64 (cols 64:128 zero), w1B covers blocks
    # 2-3 -> rows 64:128; the two matmuls accumulate into one bank.
    # w1A|w1B|chunk0|tail-chunk ride ONE first transfer (xw0): the first
    # compute waits a single DMA round trip, and the tiny tail chunk is
    # computed right after chunk 0 so the pipeline END gates on x3's
    # earlier semaphore instead of a trailing 32KB transfer.
    XW0W = 2 * P + 2 * MM + 2 * M_TAIL
    xw0 = nc.dram_tensor("xw0", [P, 2, XW0W], FP8, kind="ExternalInput")
    xt1 = nc.dram_tensor("xt1", [P, 8, MM], FP8, kind="ExternalInput")
    xt2 = nc.dram_tensor("xt2", [P, 8, MM], FP8, kind="ExternalInput")
    xt3 = nc.dram_tensor("xt3", [P, 4, MM], FP8, kind="ExternalInput")
    wpack = nc.dram_tensor("wpack", [P, NWP], DT, kind="ExternalInput")
    wcol = nc.dram_tensor("wcol", [F_COL, NWC], DT, kind="ExternalInput")
    bias32 = nc.dram_tensor("bias32", [P, NB32], f32, kind="ExternalInput")
    ident32 = nc.dram_tensor("ident32", [P, P], f32, kind="ExternalInput")
    out_a = nc.dram_tensor("out_a", [N_CHUNKS, P], f32,
                           kind="ExternalOutput")
    out_b = nc.dram_tensor("out_b", [1, P], f32, kind="ExternalOutput")

    with tile.TileContext(nc) as tc:
        with (
            tc.tile_pool(name="consts", bufs=1) as consts,
            tc.tile_pool(name="xin", bufs=1) as xin,
            tc.tile_pool(name="work", bufs=1) as work,
            tc.tile_pool(name="psum", bufs=1, space="PSUM") as psum,
        ):
            # --- input DMAs: ONE queue, strict need-order -------------
            # Parallel queues round-robin per packet, so they dilute
            # each other and the first-needed transfer lands almost as
            # late as the last.  A single FIFO queue completes in
            # consumption order at full bandwidth instead.
            # sync queue: the bulk x stream only, strict need-order (a
            # second bulk queue dilutes packet round-robin and delays
            # the first-needed transfer; constants ride scalar instead).
            x0 = xin.tile([P, 2, XW0W], FP8, tag="x0", name="x0")
            nc.sync.dma_start(x0[:, :, :], xw0[:, :, :])
            x1 = xin.tile([P, 8, MM], FP8, tag="x1", name="x1")
            nc.sync.dma_start(x1[:, :, :], xt1[:, :, :])
            x2 = xin.tile([P, 8, MM], FP8, tag="x2", name="x2")
            nc.sync.dma_start(x2[:, :, :], xt2[:, :, :])
            x3 = xin.tile([P, 4, MM], FP8, tag="x3", name="x3")
            nc.sync.dma_start(x3[:, :, :], xt3[:, :, :])
            bs = consts.tile([P, NB32], f32)
            nc.scalar.dma_start(bs[:], bias32[:])
            wp = consts.tile([P, NWP], DT)
            nc.scalar.dma_start(wp[:], wpack[:])
            # +1 spare col: DVE touches it mid-loop so the scheduler
            # cannot hoist the col matmul (which reads this tile) ahead
            # of the node chunks.
            wc = consts.tile([F_COL, NWC + 1], DT)
            nc.scalar.dma_start(wc[:, 0:NWC], wcol[:])
            nc.vector.memset(wc[:, NWC:NWC + 1], 0.0)
            i128t = consts.tile([P, P], f32)
            nc.scalar.dma_start(i128t[:], ident32[:])
            w1s = x0  # stationaries live in the head of xw0

            w2_t = wp[:, 0:P]
            cw1_t = wc[:, CW1_OFF:CW1_OFF + H]
            colT_t = wc[:, COLT_OFF:COLT_OFF + COLN]
            i128 = i128t[:, :]
            b1_t = bs[:, B1_OFF:B1_OFF + 1]
            b2_t = bs[:, B2_OFF:B2_OFF + 1]
            nb2_t = bs[:, NB2_OFF:NB2_OFF + 1]
            cb1_t = bs[:H, CB1_OFF:CB1_OFF + 1]

            # stats: cols 0:6 = first six chunks, col 6 = col-path
            # (rows 0-15; rest zeroed), col 7 = LAST chunk.  Cols 0:7
            # transpose early (hidden under last-chunk compute); only
            # col 7 transposes on the critical tail.
            stats = work.tile([P, N_CHUNKS + 1], f32)
            nc.vector.memset(stats[:, N_CHUNKS - 1:N_CHUNKS], 0.0)

            NBUF = 3
            ps1_t = [psum.tile([P, MM], f32, tag=f"ps1_{k}", name=f"ps1_{k}")
                     for k in range(NBUF)]
            ps2_t = [psum.tile([P, MM], f32, tag=f"ps2_{k}", name=f"ps2_{k}")
                     for k in range(2)]
            h1_t = [work.tile([P, MM], DT, tag=f"h1_{k}", name=f"h1_{k}")
                    for k in range(NBUF)]
            scr_t = [work.tile([P, MM], DT, tag=f"scr_{k}", name=f"scr_{k}")
                     for k in range(NBUF)]

            # emission order: chunk0, tail chunk (both ride xw0), then
            # the stream chunks; -> (rhs A slice, rhs B slice, M)
            W0 = 2 * P             # chunk0 data offset inside xw0
            WT = W0 + 2 * MM       # tail chunk offset inside xw0
            srcs = [
                (x0[:, 0:2, W0:W0 + MM],
                 x0[:, 0:2, W0 + MM:W0 + 2 * MM], MM),
                (x0[:, 0:2, WT:WT + M_TAIL],
                 x0[:, 0:2, WT + M_TAIL:WT + 2 * M_TAIL], M_TAIL),
                (x1[:, 0:2, :], x1[:, 2:4, :], MM),
                (x1[:, 4:6, :], x1[:, 6:8, :], MM),
                (x2[:, 0:2, :], x2[:, 2:4, :], MM),
                (x2[:, 4:6, :], x2[:, 6:8, :], MM),
                (x3[:, 0:2, :], x3[:, 2:4, :], MM),
            ]

            for c, (rhsA, rhsB, M) in enumerate(srcs):
                ps1 = ps1_t[c % NBUF]
                nc.tensor.matmul(
                    ps1[:, :M], w1s[:, :, 0:P], rhsA,
                    start=True, stop=False, perf_mode=DR)
                nc.tensor.matmul(
                    ps1[:, :M], w1s[:, :, P:2 * P], rhsB,
                    start=False, stop=True, perf_mode=DR)
                h1 = h1_t[c % NBUF]
                nc.scalar.activation(h1[:, :M], ps1[:, :M], Relu, bias=b1_t)
                ps2 = ps2_t[c % 2]
                nc.tensor.matmul(ps2[:, :M], w2_t, h1[:, :M],
                                 start=True, stop=True)
                scr = scr_t[c % NBUF]
                # relu(x + b2) = max(x, -b2) + b2 -- the DVE's second ALU
                # stage does not apply `max`, so keep max in stage 0.
                sc = c if c < len(srcs) - 1 else N_CHUNKS  # last -> col 7
                nc.vector.tensor_scalar(
                    scr[:, :M], ps2[:, :M], nb2_t, b2_t,
                    mybir.AluOpType.max, mybir.AluOpType.add,
                    accum_out=stats[:, sc:sc + 1])
                if c == 2:
                    # pin the col matmul behind chunk 2: write the spare
                    # col its rhs covers (junk col excluded from accum).
                    nc.vector.tensor_copy(wc[0:1, NWC:NWC + 1],
                                          stats[0:1, 2:3])

            # column-features path: h = relu(colT.T @ col_W1 + col_b1).
            # rhs covers one junk col (the scheduling pin); the accum
            # reads only the first COLN psum columns.
            psc = psum.tile([H, COLN + 1], f32, tag="psc")
            nc.tensor.matmul(psc[:, :], cw1_t,
                             wc[:, COLT_OFF:COLT_OFF + COLN + 1],
                             start=True, stop=True)
            colscr = work.tile([H, COLN], f32)
            nc.scalar.activation(colscr[:], psc[:, 0:COLN], Relu, bias=cb1_t,
                                 accum_out=stats[:H, N_CHUNKS - 1:N_CHUNKS])

            # transpose + ship cols 0:7 early (overlapping last-chunk
            # compute, including the out_a DMA receipt); only col 7
            # (the last chunk) rides the critical tail, via a second
            # transpose at psum partition 32 (dst must be 32-aligned).
            ptr = psum.tile([N_CHUNKS, P], f32, tag="ptr")
            ptr2 = psum.tile([1, P], f32, tag="ptr2")
            rowa = work.tile([N_CHUNKS, P], f32)
            rowb = work.tile([1, P], f32)
            nc.tensor.transpose(ptr[:, :], stats[:, 0:N_CHUNKS], i128)
            nc.vector.tensor_copy(rowa[:, :], ptr[:, :])
            # out_a rides the (idle) scalar queue so its dispatch does
            # not occupy the sync engine right before out_b's dispatch.
            nc.scalar.dma_start(out_a[:], rowa[:])
            nc.tensor.transpose(ptr2[:, :],
                                stats[:, N_CHUNKS:N_CHUNKS + 1], i128)
            nc.vector.tensor_copy(rowb[:, :], ptr2[:, :])
            nc.sync.dma_start(out_b[:], rowb[:])

    nc.finalize()

    if CHECK_WAITS:
        for blk in nc.m.functions[0].blocks:
            for inst in blk.instructions:
                si = inst.sync_info
                nwait = len(si.on_wait) if si and si.on_wait else 0
                limit = 2 if type(inst).__name__ in (
                    "InstEventSemaphore", "InstDrain", "InstDMACopy") else 1
                assert nwait <= limit, (
                    inst.name, type(inst).__name__,
                    [w.ant_name for w in si.on_wait])
    return nc


def _get_nc():
    if "nc" not in _NC_CACHE:
        _NC_CACHE["nc"] = _build_nc()
    return _NC_CACHE["nc"]


def _pack_x_core(xc):
    """xc [25000, 64] f32 -> [128, 12544] fp8 in chunked DR layout."""
    if N_PAD:
        xc = np.concatenate(
            [xc, np.zeros((N_PAD, F_NODE), xc.dtype)], axis=0)
    cols = []
    base = 0
    for M in CHUNK_M:
        nodes = xc[base:base + 8 * M]                 # [8M, 64]
        a = nodes.reshape(4, 2, M, F_NODE)            # (hb, prow, m, feat)
        cols.append(a.transpose(1, 3, 0, 2).reshape(P, 4 * M))
        base += 8 * M
    return np.concatenate(cols, axis=1)


def _prep_in_maps(node_features, col_features, W1, b1, W2, b2, col_W1, col_b1):
    f32 = np.float32
    x = np.ascontiguousarray(node_features, dtype=f32).reshape(B * N, F_NODE)
    colf = np.ascontiguousarray(col_features, dtype=f32).reshape(B * C, F_COL)
    W1 = np.asarray(W1, f32)
    W2 = np.asarray(W2, f32)

    # DoubleRow stationaries: w1X[p, k, 16q+f] = W1[p%64, f] where
    # q = 2*k + p//64 (4 nodes per moving column pair); w1A fills
    # out rows 0:64 (blocks 0-1), w1B rows 64:128 (blocks 2-3).
    w1dr = np.zeros((P, 2, 2 * P), f32)
    for k in range(2):
        for ph in range(2):
            q = 2 * k + ph
            w1dr[ph * 64:(ph + 1) * 64, k, 16 * q:16 * q + H] = W1          # A
            w1dr[ph * 64:(ph + 1) * 64, k, P + 64 + 16 * q:P + 64 + 16 * q + H] = W1  # B
    w1dr = w1dr.astype(NPF8)

    wpack = np.zeros((P, NWP), f32)
    for g in range(P // H):
        wpack[H * g:H * g + H, H * g:H * g + H] = W2
    wpack = wpack.astype(NPBF)

    wcol_base = np.zeros((F_COL, NWC), f32)
    wcol_base[:, CW1_OFF:CW1_OFF + H] = np.asarray(col_W1, f32)

    bias32 = np.zeros((P, NB32), f32)
    bias32[:, B1_OFF] = np.tile(np.asarray(b1, f32), P // H)
    bias32[:, B2_OFF] = np.tile(np.asarray(b2, f32), P // H)
    bias32[:, NB2_OFF] = -bias32[:, B2_OFF]
    bias32[:H, CB1_OFF] = np.asarray(col_b1, f32)
    ident32 = np.eye(P, dtype=f32)

    bounds = np.cumsum([0] + [4 * M for M in CHUNK_M])  # chunk col offsets

    in_maps = []
    for core in range(N_CORES):
        n0 = core * NODES_PER_CORE
        xp = _pack_x_core(x[n0:n0 + NODES_PER_CORE]).astype(NPF8)
        wcol = wcol_base.copy()
        wcol[:, COLT_OFF:COLT_OFF + COLN] = \
            colf[core * COLN:(core + 1) * COLN].T
        # xw0 [128, 2, 256+1024+128]: w1A | w1B | chunk0-A | chunk0-B |
        # tail-A | tail-B; dim1 is the DoubleRow k index.
        c0 = xp[:, bounds[0]:bounds[1]].reshape(P, 4, MM)
        ct = xp[:, bounds[6]:bounds[7]].reshape(P, 4, M_TAIL)
        xw0 = np.concatenate(
            [w1dr, c0[:, 0:2, :], c0[:, 2:4, :],
             ct[:, 0:2, :], ct[:, 2:4, :]], axis=2)
        in_maps.append({
            "xw0": np.ascontiguousarray(xw0),
            "xt1": xp[:, bounds[1]:bounds[3]].reshape(P, 8, MM),
            "xt2": xp[:, bounds[3]:bounds[5]].reshape(P, 8, MM),
            "xt3": xp[:, bounds[5]:bounds[6]].reshape(P, 4, MM),
            "wpack": wpack,
            "wcol": wcol.astype(NPBF),
            "bias32": bias32,
            "ident32": ident32,
        })
    return in_maps


def kernel(node_features, col_features, edge_index, W1, b1, W2, b2,
           node_fc_W, node_fc_b, col_W1, col_b1, col_W2, col_b2,
           fc_W, fc_b, out_W, out_b):
    global LAST_EXEC_TIME_NS, LAST_RESULTS
    # edge_index provably does not affect the output (see module docstring).
    in_maps = _prep_in_maps(node_features, col_features,
                            W1, b1, W2, b2, col_W1, col_b1)
    nc = _get_nc()
    res = run_bass_kernel_spmd(nc, in_maps, core_ids=list(range(N_CORES)),
                               trace=PROFILE)
    LAST_EXEC_TIME_NS = res.exec_time_ns
    LAST_RESULTS = res
    outs = res.results

    node_fc_W = np.asarray(node_fc_W, np.float32)
    col_W2 = np.asarray(col_W2, np.float32)
    # Device relu2 computes max(x, -b2) (+b2 once per chunk-reduction),
    # i.e. each chunk col = sum(relu) - (M-1)*b2; add the constant back.
    # Zero-pad nodes contribute relu(W2.T relu(b1) + b2) each; subtract.
    b2f = np.asarray(b2, np.float32)
    summ1 = np.float32(sum(M - 1 for M in CHUNK_M))
    pad_h2 = np.maximum(
        np.maximum(np.asarray(b1, np.float32), 0.0) @ np.asarray(W2, np.float32)
        + b2f, 0.0) * np.float32(N_PAD) - summ1 * b2f * np.float32(P // H)
    node_avg = np.zeros((B, 1), np.float32)
    col_avg = np.zeros((B, 1), np.float32)
    for b in range(B):
        acc = (outs[2 * b]["out_a"].astype(np.float32) +
               outs[2 * b + 1]["out_a"].astype(np.float32))
        accb = (outs[2 * b]["out_b"].astype(np.float32) +
                outs[2 * b + 1]["out_b"].astype(np.float32))
        node_rows = acc[:N_CHUNKS - 1].sum(axis=0) + accb[0]
        ns = node_rows.reshape(P // H, H).sum(axis=0) - 2.0 * pad_h2
        cs = acc[N_CHUNKS - 1, :H]                       # col feature sums
        node_avg[b, 0] = (ns / np.float32(N)) @ node_fc_W[:, 0] + \
            np.asarray(node_fc_b, np.float32)[0]
        col_avg[b, 0] = (cs / np.float32(C)) @ col_W2[:, 0] + \
            np.asarray(col_b2, np.float32)[0]

    combined = np.concatenate([node_avg, col_avg], axis=1)      # [B, 2]
    z = np.maximum(combined @ np.asarray(fc_W, np.float32) +
                   np.asarray(fc_b, np.float32), 0.0)
    out = z @ np.asarray(out_W, np.float32) + np.asarray(out_b, np.float32)
    return out.astype(np.float32)
